# revision 1
# baseline (speedup 1.0000x reference)
"""Bass/Tile TRN2 kernel for nn_AttentionLayer_78889959293198 (sparse_attention).

Pure data parallel over batch: B=4096 split across 8 NeuronCores, 512
items/core.  All matmuls in bf16 (fp32 I/O), layouts built around a
32-padded position axis so 4 items fill the 128-partition dim:

  x (host 32-padded, zeros) -> row-major x_bf -> PE-transposed XT (with
  zero guard columns) -> conv1d as 6 full-tile shifted matmuls/projection
  -> LayerNorm stats fused into the PSUM eviction (tensor_tensor_reduce)
  -> residual on gpsimd -> per-head scores via 32x32 block-transposed q/k
  (DVE stream transpose), static bias via one K=25 matmul, dynamic
  tanh-pair bias via per-(item,j) K=64 matmuls accumulated directly into
  the scores PSUM -> exp softmax without max-subtraction -> attn@v from
  stream-transposed attn -> output projection with bo as a K=1
  ones-outer-product matmul -> PSUM DMA'd straight to HBM (32-padded;
  host strips).
"""

import math
import os
import sys

import numpy as np

for _p in ("/opt/trn_rl_repo", "/root/.axon_site/_ro/trn_rl_repo"):
    if os.path.isdir(_p) and _p not in sys.path:
        sys.path.insert(0, _p)

import concourse.bass as bass  # noqa: E402
import concourse.bacc as bacc  # noqa: E402
import concourse.tile as tile  # noqa: E402
from concourse import mybir  # noqa: E402

FP32 = mybir.dt.float32
BF16 = mybir.dt.bfloat16
AF = mybir.ActivationFunctionType
ALU = mybir.AluOpType

EPS = 1e-5
H = 8
N = 25
NP = 32
D = 256
DQ = 64
DK = 32
N_CORES = 8
SUP = 16                      # items per supertile
SCALE = 1.0 / math.sqrt(DK)
RS = math.sqrt(DK)            # biases pre-scaled so exp() can fold 1/sqrt(dk)


# ---------------------------------------------------------------------------
# device program
# ---------------------------------------------------------------------------

STAGE = int(os.environ.get("K_STAGE", "9"))
SUB = int(os.environ.get("K_SUB", "9"))


def build_program(b_core, ln_trivial=True):
    n_sup = b_core // SUP
    assert n_sup * SUP == b_core
    nc = bacc.Bacc("TRN2", target_bir_lowering=False, debug=False)

    x32 = nc.dram_tensor("x32", [b_core * NP, D], FP32, kind="ExternalInput")
    w_mov = nc.dram_tensor("w_mov", [3, 2, 3, 128, D], BF16,
                           kind="ExternalInput")
    wqf_s = nc.dram_tensor("wqf_s", [2, 128, DQ], BF16, kind="ExternalInput")
    wqp_rep = nc.dram_tensor("wqp_rep", [128, H], BF16, kind="ExternalInput")
    static_f = nc.dram_tensor("static_f", [N, D], BF16, kind="ExternalInput")
    ident_sel = nc.dram_tensor("ident_sel", [N, 128], BF16,
                               kind="ExternalInput")
    id128 = nc.dram_tensor("id128", [128, 128], BF16, kind="ExternalInput")
    ones_col = nc.dram_tensor("ones_col", [1, 128], BF16,
                              kind="ExternalInput")
    bo_row = nc.dram_tensor("bo_row", [1, D], BF16, kind="ExternalInput")
    wo_m = nc.dram_tensor("wo_m", [2, 128, D], BF16, kind="ExternalInput")
    gb_rep = nc.dram_tensor("gb_rep", [6, D], BF16, kind="ExternalInput")
    y32 = nc.dram_tensor("y32", [b_core * NP, D], FP32, kind="ExternalOutput")

    with tile.TileContext(nc) as tc:
        with (
            tc.tile_pool(name="const", bufs=1) as const,
            tc.tile_pool(name="xpool", bufs=2) as xpool,
            tc.tile_pool(name="work", bufs=3) as work,
            tc.tile_pool(name="small", bufs=4) as small,
            tc.tile_pool(name="pbig", bufs=2, space="PSUM") as pbig,
            tc.tile_pool(name="ps_s", bufs=2, space="PSUM") as ps_s,
            tc.tile_pool(name="ps_o", bufs=2, space="PSUM") as ps_o,
            tc.tile_pool(name="ps_y", bufs=2, space="PSUM") as ps_y,
        ):
            C = _load_consts(nc, const, w_mov, wqf_s, wqp_rep, static_f,
                             ident_sel, id128, ones_col, bo_row, wo_m, gb_rep)
            for s in range(n_sup):
                _supertile(nc, s, C, xpool, work, small, pbig, ps_s, ps_o,
                           ps_y, x32, y32, ln_trivial)
    nc.finalize()
    return nc


def _load_consts(nc, const, w_mov, wqf_s, wqp_rep, static_f, ident_sel,
                 id128, ones_col, bo_row, wo_m, gb_rep):
    C = {}

    def ld(name, shape, dram, in_ap=None):
        t = const.tile(shape, BF16, tag=name)
        nc.sync.dma_start(t[...], dram[...] if in_ap is None else in_ap)
        C[name] = t
        return t

    ld("w", [128, 3, 2, 3, D], w_mov,
       w_mov[...].rearrange("p c t k o -> k p c t o"))
    ld("wqf", [128, 2, DQ], wqf_s, wqf_s[...].rearrange("c k m -> k c m"))
    ld("wqp", [128, H], wqp_rep)
    ld("stat", [N, D], static_f)
    ld("isel", [N, 128], ident_sel)
    ld("id128", [128, 128], id128)
    ld("ones", [1, 128], ones_col)
    ld("bo", [1, D], bo_row)
    ld("wo", [128, 2, D], wo_m, wo_m[...].rearrange("c k o -> k c o"))
    ld("gb", [6, D], gb_rep)
    zrow = const.tile([1, D], BF16, tag="zrow", name="zrow")
    nc.gpsimd.memset(zrow[...], 0.0)
    C["zrow"] = zrow
    z128 = const.tile([128, D], BF16, tag="z128", name="z128")
    nc.gpsimd.memset(z128[...], 0.0)
    C["z128"] = z128
    return C


def _supertile(nc, s, C, xpool, work, small, pbig, ps_s, ps_o, ps_y,
               x32, y32, ln_trivial):
    # ---- load + cast x ----
    x_raw = xpool.tile([128, 4, D], FP32, tag="x_raw", name="x_raw")
    nc.sync.dma_start(
        x_raw[...],
        x32[s * SUP * NP:(s + 1) * SUP * NP, :]
        .rearrange("(g a i) d -> (a i) g d", g=4, a=4))
    x_bf = xpool.tile([128, 4, D], BF16, tag="x_bf", name="x_bf")
    nc.gpsimd.tensor_copy(x_bf[...], x_raw[...])

    # ---- XT (feature-major x) via PE transpose; 1+512+1 guard columns ----
    xt = [xpool.tile([128, 514], BF16, tag=f"xt{c}", name=f"xt{c}") for c in range(2)]
    for c in range(2):
        nc.vector.memset(xt[c][:, 0:514:513], 0.0)
    for g in range(4):
        for c in range(2):
            ptr = pbig.tile([128, 128], BF16, tag="cbank", name="ptr")
            nc.tensor.transpose(ptr[:, 0:128],
                                x_bf[:, g:g + 1, c * 128:(c + 1) * 128],
                                C["id128"][...])
            nc.vector.tensor_copy(
                xt[c][:, 1 + g * 128:1 + (g + 1) * 128], ptr[:, 0:128])

    qkv = [work.tile([128, 4, D], BF16, tag=f"qkv{p}", name=f"qkv{p}") for p in range(3)]

    if STAGE <= 2:
        src_t = x_bf if STAGE == 1 else None
        for pair in range(2):
            y_sb = work.tile([128, 2, D], FP32, tag="y_sb", name="y_sb")
            if STAGE == 1:
                nc.vector.tensor_copy(y_sb[...], x_bf[:, 2 * pair:2 * pair + 2, :])
            else:
                nc.vector.tensor_copy(
                    y_sb[...],
                    xt[pair][:, 1:513].rearrange("p (sl d) -> p sl d", sl=2))
            r0 = (s * SUP + pair * 8) * NP
            nc.sync.dma_start(
                y32[r0:r0 + 8 * NP, :]
                .rearrange("(sl a i) d -> (a i) sl d", sl=2, a=4),
                y_sb[...])
        return

    for g in range(4):
        _group(nc, s, g, C, work, small, pbig, ps_s, ps_o, ps_y,
               x_bf, xt, qkv, y32, ln_trivial)


def _group(nc, s, g, C, work, small, pbig, ps_s, ps_o, ps_y, x_bf, xt, qkv,
           y32, ln_trivial):
    # ---------------- conv + LN + residual ----------------
    sC = small.tile([128, 3], FP32, tag="sC", name="sC")
    sQ = small.tile([128, 3], FP32, tag="sQ", name="sQ")
    c_bf = [small.tile([128, D], BF16, tag=f"cbf{p}", name=f"cbf{p}") for p in range(3)]
    sq_junk = small.tile([128, D], BF16, tag="sqj", name="sqj")

    for p in range(3):
        pc = pbig.tile([128, D], FP32, tag="cbank", name="cbank")
        seq = [(0, 1), (0, 0), (0, 2), (1, 0), (1, 2), (1, 1)]
        for idx, (c, tap) in enumerate(seq):
            lhs = xt[c][:, 1 + g * 128 + (tap - 1):
                        1 + g * 128 + (tap - 1) + 128]
            nc.tensor.matmul(pc[...], lhs, C["w"][:, p:p + 1, c:c + 1,
                                                  tap:tap + 1, :],
                             start=(idx == 0), stop=(idx == len(seq) - 1),
                             skip_group_check=True)
        nc.vector.tensor_copy(c_bf[p][...], pc[...])
        nc.vector.reduce_sum(sC[:, p:p + 1], c_bf[p][...],
                             axis=mybir.AxisListType.X)
        nc.vector.tensor_tensor(sq_junk[...], c_bf[p][...], c_bf[p][...],
                                op=ALU.mult)
        nc.vector.reduce_sum(sQ[:, p:p + 1], sq_junk[...],
                             axis=mybir.AxisListType.X)

    mu = small.tile([128, 3], FP32, tag="mu", name="mu")
    nc.vector.tensor_scalar_mul(mu[...], sC[...], 1.0 / D)
    qbar = small.tile([128, 3], FP32, tag="qbar", name="qbar")
    nc.vector.tensor_scalar(qbar[...], sQ[...], 1.0 / D, EPS,
                            op0=ALU.mult, op1=ALU.add)
    mu2 = small.tile([128, 3], FP32, tag="mu2", name="mu2")
    nc.vector.tensor_tensor(mu2[...], mu[...], mu[...], op=ALU.mult)
    var = small.tile([128, 3], FP32, tag="var", name="var")
    nc.vector.tensor_tensor(var[...], qbar[...], mu2[...], op=ALU.subtract)
    sig = small.tile([128, 3], FP32, tag="sig", name="sig")
    nc.scalar.sqrt(sig[...], var[...])
    isig = small.tile([128, 3], FP32, tag="isig", name="isig")
    nc.vector.reciprocal(isig[...], sig[...])
    nmus = small.tile([128, 3], FP32, tag="nmus", name="nmus")
    nc.vector.tensor_tensor(nmus[...], mu[...], isig[...], op=ALU.mult)

    for p in range(3):
        t1 = small.tile([128, D], BF16, tag="t1", name="t1")
        nc.vector.tensor_scalar(t1[...], c_bf[p][...], isig[:, p:p + 1],
                                nmus[:, p:p + 1], op0=ALU.mult,
                                op1=ALU.subtract)
        dst = qkv[p][:, g:g + 1, :]
        if ln_trivial:
            nc.gpsimd.tensor_tensor(dst, t1[...].unsqueeze(1),
                                    x_bf[:, g:g + 1, :], op=ALU.add)
        else:
            t2 = small.tile([128, D], BF16, tag="t2", name="t2")
            nc.gpsimd.tensor_tensor(
                t2[...], t1[...],
                C["gb"][2 * p:2 * p + 1, :].broadcast_to([128, D]),
                op=ALU.mult)
            t3 = small.tile([128, D], BF16, tag="t3", name="t3")
            nc.gpsimd.tensor_tensor(
                t3[...], t2[...],
                C["gb"][2 * p + 1:2 * p + 2, :].broadcast_to([128, D]),
                op=ALU.add)
            nc.gpsimd.tensor_tensor(dst, t3[...].unsqueeze(1),
                                    x_bf[:, g:g + 1, :], op=ALU.add)

    if STAGE <= 3:
        if g % 2 == 1:
            y_sb = work.tile([128, 2, D], FP32, tag="y_sb", name="y_sb")
            nc.vector.tensor_copy(y_sb[:, 0:1, :], qkv[0][:, g - 1:g, :])
            nc.vector.tensor_copy(y_sb[:, 1:2, :], qkv[1][:, g:g + 1, :])
            r0 = (s * SUP + (g - 1) * 4) * NP
            nc.sync.dma_start(
                y32[r0:r0 + 8 * NP, :]
                .rearrange("(sl a i) d -> (a i) sl d", sl=2, a=4),
                y_sb[...])
        return

    # ---------------- qf + pair tanh ----------------
    pqf = pbig.tile([128, 2, NP], FP32, tag="cbank", name="cbank")
    for t in range(2):
        for par in range(2):
            a = 2 * t + par
            for c in range(2):
                nc.tensor.matmul(
                    pqf[64 * par:64 * par + 64, t:t + 1, :],
                    C["wqf"][:, c:c + 1, :],
                    xt[c][:, 1 + g * 128 + a * 32:1 + g * 128 + a * 32 + 32],
                    start=(c == 0), stop=(c == 1), skip_group_check=True)
    qft = small.tile([128, 2, N], BF16, tag="qft", name="qft")
    nc.vector.tensor_copy(qft[...], pqf[:, :, 0:N])

    dd = work.tile([128, 2, N, N], BF16, tag="dd", name="dd")
    for t in range(2):
        in0 = qft[:, t:t + 1, :].unsqueeze(3).broadcast_to([128, 1, N, N])
        in1 = qft[:, t:t + 1, :].unsqueeze(2).broadcast_to([128, 1, N, N])
        eng = nc.vector if t == 0 else nc.gpsimd
        eng.tensor_tensor(dd[:, t:t + 1, :, :], in0, in1, op=ALU.subtract)
    pp = work.tile([128, 2, N, N], BF16, tag="pp", name="pp")
    nc.scalar.activation(pp[...], dd[...], AF.Tanh)

    if STAGE <= 4:
        if g % 2 == 1:
            y_sb = work.tile([128, 2, D], FP32, tag="y_sb", name="y_sb")
            nc.vector.tensor_copy(
                y_sb[...], pp[:, 0:1, :, :].rearrange("p t (x d) -> p (t x) d", x=2)[:, :, 0:D])
            r0 = (s * SUP + (g - 1) * 4) * NP
            nc.sync.dma_start(
                y32[r0:r0 + 8 * NP, :]
                .rearrange("(sl a i) d -> (a i) sl d", sl=2, a=4),
                y_sb[...])
        return

    # ---------------- scores ----------------
    qt_blk = work.tile([128, H, 32], BF16, tag="qt_blk", name="qt_blk")
    kt_blk = work.tile([128, H, 32], BF16, tag="kt_blk", name="kt_blk")
    nc.vector.transpose(qt_blk[...], qkv[0][:, g:g + 1, :])
    nc.vector.transpose(kt_blk[...], qkv[1][:, g:g + 1, :])

    ps = ps_s.tile([128, H, 32], FP32, tag="sbank", name="sbank")
    nc.tensor.matmul(ps[...], C["isel"][...], C["stat"][...],
                     start=True, stop=False, skip_group_check=True)
    for h in range(H):
        for a in range(4):
            nc.tensor.matmul(
                ps[32 * a:32 * a + N, h:h + 1, 0:N],
                qt_blk[32 * a:32 * a + 32, h:h + 1, 0:N],
                kt_blk[32 * a:32 * a + 32, h:h + 1, 0:N],
                start=False, stop=False, skip_group_check=True,
                tile_position=(32 * a, 32 * a))
    for j in range(N):
        for t in range(2):
            for par in range(2):
                a = 2 * t + par
                nc.tensor.matmul(
                    ps[32 * a:32 * a + N, :, j:j + 1],
                    pp[64 * par:64 * par + 64, t:t + 1, :, j:j + 1],
                    C["wqp"][64 * par:64 * par + 64, :],
                    start=False, stop=False, skip_group_check=True,
                    tile_position=(64 * par, 32 * a))
    nc.tensor.matmul(ps[...], C["ones"][...], C["zrow"][...],
                     start=False, stop=True, skip_group_check=True)

    # ---------------- softmax (no max subtraction) ----------------
    attn_u = work.tile([128, H, 32], BF16, tag="attn_u", name="attn_u")
    nc.scalar.activation(attn_u[...], ps[...], AF.Exp, scale=SCALE)
    den = small.tile([128, H], FP32, tag="den", name="den")
    nc.vector.reduce_sum(den[...], attn_u[:, :, 0:N],
                         axis=mybir.AxisListType.X)
    rden = small.tile([128, H], FP32, tag="rden", name="rden")
    nc.vector.reciprocal(rden[...], den[...])
    attn_t = work.tile([128, H, 32], BF16, tag="attn_t", name="attn_t")
    nc.vector.transpose(attn_t[...], attn_u[...])

    if STAGE <= 5:
        if g % 2 == 1:
            y_sb = work.tile([128, 2, D], FP32, tag="y_sb", name="y_sb")
            nc.vector.tensor_copy(y_sb[:, 0:1, :],
                                  attn_t[...].rearrange("p h j -> p (h j)").unsqueeze(1))
            nc.vector.tensor_copy(y_sb[:, 1:2, :],
                                  attn_u[...].rearrange("p h j -> p (h j)").unsqueeze(1))
            r0 = (s * SUP + (g - 1) * 4) * NP
            nc.sync.dma_start(
                y32[r0:r0 + 8 * NP, :]
                .rearrange("(sl a i) d -> (a i) sl d", sl=2, a=4),
                y_sb[...])
        return

    # ---------------- attn @ v ----------------
    po = ps_o.tile([128, H, 32], FP32, tag="obank", name="obank")
    for h in range(H):
        for a in range(4):
            nc.tensor.matmul(
                po[32 * a:32 * a + 32, h:h + 1, :],
                attn_t[32 * a:32 * a + 32, h:h + 1, :],
                qkv[2][32 * a:32 * a + 32, g:g + 1, 32 * h:32 * h + 32],
                skip_group_check=True,
                tile_position=(32 * a, 32 * a))
    o_bf = work.tile([128, H, 32], BF16, tag="o_bf", name="o_bf")
    nc.vector.tensor_tensor(o_bf[...], po[...],
                            rden[...].unsqueeze(2).broadcast_to([128, H, 32]),
                            op=ALU.mult)
    # O row-major [(a,i), (h,d)] -> feature-major via PE transpose
    ot_bf = [work.tile([128, 128], BF16, tag=f"ot{c}", name=f"ot{c}")
             for c in range(2)]
    for c in range(2):
        ptc = pbig.tile([128, 128], BF16, tag="cbank", name="ptc")
        nc.tensor.transpose(
            ptc[...],
            o_bf[:, 4 * c:4 * (c + 1), :].rearrange("p h d -> p (h d)"),
            C["id128"][...])
        nc.vector.tensor_copy(ot_bf[c][...], ptc[...])
    if SUB <= 6:
        if g % 2 == 1:
            y_sb = work.tile([128, 2, D], FP32, tag="y_sb", name="y_sb")
            nc.vector.tensor_copy(
                y_sb[:, 0:1, :],
                o_bf[...].rearrange("p h d -> p (h d)").unsqueeze(1))
            nc.vector.memset(y_sb[:, 1:2, :], 0.0)
            r0 = (s * SUP + (g - 1) * 4) * NP
            nc.sync.dma_start(
                y32[r0:r0 + 8 * NP, :]
                .rearrange("(sl a i) d -> (a i) sl d", sl=2, a=4),
                y_sb[...])
        return

    # ---------------- output projection ----------------
    slot = g % 2
    if slot == 0:
        py = ps_y.tile([128, 2, D], FP32, tag="ybank", name="ybank")
        _group.py_tile = py
    else:
        py = _group.py_tile
    use_bo = SUB > 7
    if use_bo:
        nc.tensor.matmul(py[:, slot:slot + 1, :], C["ones"][...],
                         C["bo"][...], start=True, stop=False,
                         skip_group_check=True)
    for c in range(2):
        nc.tensor.matmul(py[:, slot:slot + 1, :], ot_bf[c][...],
                         C["wo"][:, c:c + 1, :],
                         start=(c == 0 and not use_bo), stop=(c == 1),
                         skip_group_check=True)
    if slot == 1:
        y_sb = work.tile([128, 2, D], FP32, tag="y_sb", name="y_sb")
        if g == 1:
            nc.vector.tensor_copy(y_sb[...], py[...])
        else:
            nc.scalar.copy(y_sb[...], py[...])
        r0 = (s * SUP + (g - 1) * 4) * NP
        nc.sync.dma_start(
            y32[r0:r0 + 8 * NP, :]
            .rearrange("(sl a i) d -> (a i) sl d", sl=2, a=4),
            y_sb[...])


# ---------------------------------------------------------------------------
# host wrapper
# ---------------------------------------------------------------------------

_CACHE = {}


def _prep_consts(wq, wk, wv, rel_table, global_bias, alpha, wqf, wqp, bqp,
                 wo, bo, ln_g, ln_b):
    bf = np.float32  # cast to bf16 via ml_dtypes at the end
    import ml_dtypes
    b16 = ml_dtypes.bfloat16

    w_mov = np.zeros([3, 2, 3, 128, D], bf)
    for p, w in enumerate((wq, wk, wv)):
        for c in range(2):
            for t in range(3):
                w_mov[p, c, t] = w[:, 128 * c:128 * (c + 1), t].T
    wqf_s = np.stack([wqf[0:128], wqf[128:256]])
    wqp_rep = np.tile(wqp * RS, (2, 1))

    ids = np.arange(N)
    rel_idx = ids[:, None] - ids[None, :] + N - 1
    rel_bias = rel_table[rel_idx].transpose(2, 0, 1)          # [H, N, N]
    static = (rel_bias + global_bias * np.float32(alpha)) * RS
    static_f = np.full([N, D], -3000.0, bf)
    for h in range(H):
        static_f[:, 32 * h:32 * h + N] = static[h]            # [i, (h, j)]
    # dyn bias bqp: constant over (i,j,h)? no - per h; fold into static
    static_f[:, [32 * h + j for h in range(H) for j in range(N)]] += 0  # noop
    if np.any(bqp != 0):
        for h in range(H):
            static_f[:, 32 * h:32 * h + N] += bqp[h] * RS

    ident_sel = np.zeros([N, 128], bf)
    for a in range(4):
        ident_sel[np.arange(N), 32 * a + np.arange(N)] = 1.0
    id128 = np.eye(128, dtype=bf)
    ones_col = np.ones([1, 128], bf)
    bo_row = bo.reshape(1, D)
    wo_m = np.stack([wo[0:128], wo[128:256]])
    gb_rep = np.stack([ln_g[0], ln_b[0], ln_g[1], ln_b[1], ln_g[2], ln_b[2]])

    consts = dict(w_mov=w_mov, wqf_s=wqf_s, wqp_rep=wqp_rep,
                  static_f=static_f, ident_sel=ident_sel, id128=id128,
                  ones_col=ones_col, bo_row=bo_row, wo_m=wo_m, gb_rep=gb_rep)
    return {k: np.asarray(v, b16) for k, v in consts.items()}


def kernel(x, wq, wk, wv, ln_q_g, ln_q_b, ln_k_g, ln_k_b, ln_v_g, ln_v_b,
           rel_table, global_bias, alpha, wqf, bqf, wqp, bqp, wo, bo):
    x = np.asarray(x, np.float32)
    B = x.shape[0]
    b_core = B // N_CORES
    assert np.allclose(np.asarray(bqf), 0.0), "bqf folds out of pair diffs"

    ln_g = [np.asarray(v, np.float32) for v in (ln_q_g, ln_k_g, ln_v_g)]
    ln_b = [np.asarray(v, np.float32) for v in (ln_q_b, ln_k_b, ln_v_b)]
    ln_trivial = all(np.allclose(g, 1.0) for g in ln_g) and \
        all(np.allclose(b, 0.0) for b in ln_b)

    key = (b_core, ln_trivial)
    if key not in _CACHE:
        _CACHE[key] = build_program(b_core, ln_trivial)
    nc = _CACHE[key]

    consts = _prep_consts(
        np.asarray(wq, np.float32), np.asarray(wk, np.float32),
        np.asarray(wv, np.float32), np.asarray(rel_table, np.float32),
        np.asarray(global_bias, np.float32), np.float32(alpha),
        np.asarray(wqf, np.float32), np.asarray(wqp, np.float32),
        np.asarray(bqp, np.float32), np.asarray(wo, np.float32),
        np.asarray(bo, np.float32), ln_g, ln_b)

    # 32-pad positions, shard across cores
    x32 = np.zeros([B, NP, D], np.float32)
    x32[:, :N] = x
    x32 = x32.reshape(N_CORES, b_core * NP, D)

    in_maps = []
    for cidx in range(N_CORES):
        m = {"x32": x32[cidx]}
        m.update({k if k != "w_mov" else "w_mov": v
                  for k, v in consts.items()})
        # tensor names in program: w_mov, wqf_s, wqp_rep, static_f,
        # ident_sel, id128, ones_col, bo_row, wo_m, gb_rep
        in_maps.append(m)

    run = _get_runner(key, nc)
    outs = run(in_maps)
    outs = [o.reshape(b_core, NP, D)[:, :N] for o in outs]
    return np.concatenate(outs, axis=0).astype(np.float32)


_RUNNERS = {}


def _get_runner(key, nc):
    """Build (once) a jitted 8-core SPMD executor for `nc`.

    Mirrors bass2jax.run_bass_via_pjrt's multi-core path, but caches the
    jitted callable so repeated kernel() calls skip retracing, and skips
    output-donation so timing runs can reuse device-resident buffers.
    """
    if key in _RUNNERS:
        return _RUNNERS[key]
    import jax
    from jax.sharding import Mesh, PartitionSpec
    from jax.experimental.shard_map import shard_map
    from concourse import bass2jax, mybir as mb

    bass2jax.install_neuronx_cc_hook()
    partition_name = (nc.partition_id_tensor.name
                      if nc.partition_id_tensor else None)
    in_names, out_names, out_avals, zero_outs = [], [], [], []
    for alloc in nc.m.functions[0].allocations:
        if not isinstance(alloc, mb.MemoryLocationSet):
            continue
        name = alloc.memorylocations[0].name
        if alloc.kind == "ExternalInput":
            if name != partition_name:
                in_names.append(name)
        elif alloc.kind == "ExternalOutput":
            shape = tuple(alloc.tensor_shape)
            dtype = mb.dt.np(alloc.dtype)
            out_names.append(name)
            out_avals.append(jax.core.ShapedArray(shape, dtype))
            zero_outs.append(np.zeros(shape, dtype))
    n_params = len(in_names)
    all_names = list(in_names) + list(out_names)
    if partition_name is not None:
        all_names.append(partition_name)

    def _body(*args):
        operands = list(args)
        if partition_name is not None:
            operands.append(bass2jax.partition_id_tensor())
        return tuple(bass2jax._bass_exec_p.bind(
            *operands, out_avals=tuple(out_avals),
            in_names=tuple(all_names), out_names=tuple(out_names),
            lowering_input_output_aliases=(), sim_require_finite=True,
            sim_require_nnan=True, nc=nc))

    devices = jax.devices()[:N_CORES]
    mesh = Mesh(np.asarray(devices), ("core",))
    n_ops = n_params + len(out_names)
    fn = jax.jit(shard_map(
        _body, mesh=mesh, in_specs=(PartitionSpec("core"),) * n_ops,
        out_specs=(PartitionSpec("core"),) * len(out_names),
        check_rep=False), keep_unused=True)

    state = {"dev_consts": None}

    def run(in_maps, timing_reps=0):
        import jax as _jax
        concat = []
        for i, name in enumerate(in_names):
            concat.append(np.concatenate(
                [np.asarray(in_maps[c][name]) for c in range(N_CORES)],
                axis=0))
        concat += [np.zeros((N_CORES * z.shape[0], *z.shape[1:]), z.dtype)
                   for z in zero_outs]
        out = fn(*concat)
        _jax.block_until_ready(out)
        if timing_reps:
            dev = [_jax.device_put(a) for a in concat]
            _jax.block_until_ready(dev)
            best = float("inf")
            import time as _t
            for _ in range(timing_reps):
                t0 = _t.perf_counter()
                o = fn(*dev)
                _jax.block_until_ready(o)
                best = min(best, _t.perf_counter() - t0)
            run.last_exec_s = best
        y = np.asarray(out[0])
        per = y.shape[0] // N_CORES
        return [y[c * per:(c + 1) * per] for c in range(N_CORES)]

    _RUNNERS[key] = run
    return run


# ---------------------------------------------------------------------------
# dev: simulator check on a small shard
# ---------------------------------------------------------------------------

def _patch_sim_strided_matmul():
    """CoreSim's PSUM zero-region model only handles contiguous matmul
    out free-APs.  For strided outs (our dyn-bias accumulation) the HW
    per-element has_written semantics reduce to pure accumulation, since
    the bytes were started+written by the preceding full-tile matmul."""
    import concourse.bass_interp as bi
    import numpy as _np
    cls = bi.InstructionExecutor
    if getattr(cls, "_strided_mm_patched", False):
        return
    orig = cls.visit_InstMatmult

    def visit(self, instruction, *, reg_snapshot=None):
        out = instruction.outs[0]
        ap = [d for d in out.ap[1:] if d[1] > 1]
        contig = (not ap) or ap[-1][0] == 1
        if contig:
            return orig(self, instruction, reg_snapshot=reg_snapshot)
        assert not instruction.start_tensor_calc
        assert not instruction.stop_tensor_calc
        from concourse.bass_interp import Direction
        ifmap = self.view_ap(instruction.ins[0], Direction.READ, instruction,
                             reg_snapshot=reg_snapshot)
        weights = self.view_ap(instruction.ins[1], Direction.READ,
                               instruction, reg_snapshot=reg_snapshot)
        out_view = self.view_ap(out, Direction.WRITE, instruction,
                                required_byte_align=4,
                                reg_snapshot=reg_snapshot)
        i = ifmap.astype(_np.float32).reshape(ifmap.shape[0], -1)
        w = weights.astype(_np.float32).reshape(weights.shape[0], -1)
        out_view[:] += (w.T @ i).reshape(out_view.shape)

    cls.visit_InstMatmult = visit
    cls._strided_mm_patched = True


def _sim_check(b_core=16):
    _patch_sim_strided_matmul()
    from concourse.bass_interp import CoreSim
    sys.path.insert(0, "/root/problem")
    import reference

    inp = {k: np.asarray(v) for k, v in reference.setup_inputs().items()}
    xs = inp["x"][:b_core]
    ln_g = [inp[f"ln_{p}_g"] for p in "qkv"]
    ln_b = [inp[f"ln_{p}_b"] for p in "qkv"]
    nc = build_program(b_core, True)
    consts = _prep_consts(inp["wq"], inp["wk"], inp["wv"], inp["rel_table"],
                          inp["global_bias"], np.float32(inp["alpha"]),
                          inp["wqf"], inp["wqp"], inp["bqp"], inp["wo"],
                          inp["bo"], ln_g, ln_b)
    x32 = np.zeros([b_core, NP, D], np.float32)
    x32[:, :N] = xs
    sim = CoreSim(nc)
    sim.tensor("x32")[:] = x32.reshape(b_core * NP, D)
    for k, v in consts.items():
        name = {"w_mov": "w_mov", "wqf_s": "wqf_s", "wqp_rep": "wqp_rep",
                "static_f": "static_f", "ident_sel": "ident_sel",
                "id128": "id128", "ones_col": "ones_col", "bo_row": "bo_row",
                "wo_m": "wo_m", "gb_rep": "gb_rep"}[k]
        sim.tensor(name)[:] = v
    sim.simulate()
    y = np.asarray(sim.tensor("y32")).reshape(b_core, NP, D)[:, :N]

    ref_in = dict(inp)
    ref_in["x"] = xs
    exp = np.asarray(reference.reference(**ref_in))
    rel = np.linalg.norm(y - exp) / np.linalg.norm(exp)
    print("sim rel err:", rel)
    return rel


if __name__ == "__main__":
    _sim_check(int(os.environ.get("SIM_B", "16")))



# revision 5
# speedup vs baseline: 52.7350x; 52.7350x over previous
"""Bass/Tile TRN2 kernel for nn_AttentionLayer_78889959293198 (sparse_attention).

Pure data parallel over batch: B=4096 split across 8 NeuronCores, 512
items/core.  Layouts built around a 32-padded position axis so 4 items
fill the 128-partition dim; supertile = 16 items.

v1 restructure vs v0 baseline (3.12 ms/core):
  - LN stats: conv weights carry an extra summed column (mean comes out of
    the matmul), sum-of-squares via one fused tensor_tensor_reduce, and the
    LN scale/shift is folded into the PSUM eviction on the scalar engine
    (per-partition scale/bias APs).  Replaces ~4 big DVE passes per
    (group, proj).
  - 1/sigma via Quake-style rsqrt (bit trick + 2 Newton steps) on DVE —
    removes scalar.sqrt and with it the ACT table thrash (sqrt and
    exp/tanh live in different table sets; v0 reloaded tables 257x).
  - qf projection: 4 supertile-wide matmuls (strided moving operand)
    instead of 32 tiny ones; pair-difference inputs read straight out of
    the evicted [128, 8, 32] tile via broadcast APs.
  - residual adds batched per (proj, supertile) on gpsimd; x cast on ACT;
    softmax denominators on gpsimd; q/k/attn 32-block transposes batched
    per supertile on DVE.
  - PSUM: two pools x 4 bufs (bank-granular slots).
"""

import math
import os
import sys

import numpy as np

for _p in ("/opt/trn_rl_repo", "/root/.axon_site/_ro/trn_rl_repo"):
    if os.path.isdir(_p) and _p not in sys.path:
        sys.path.insert(0, _p)

import concourse.bass as bass  # noqa: E402
import concourse.bacc as bacc  # noqa: E402
import concourse.tile as tile  # noqa: E402
from concourse import mybir  # noqa: E402

FP32 = mybir.dt.float32
BF16 = mybir.dt.bfloat16
UI32 = mybir.dt.uint32
AF = mybir.ActivationFunctionType
ALU = mybir.AluOpType

EPS = 1e-5
H = 8
N = 25
NP = 32
D = 256
DQ = 64
DK = 32
N_CORES = 8
SUP = 16                      # items per supertile
SCALE = 1.0 / math.sqrt(DK)
RS = math.sqrt(DK)            # biases pre-scaled so exp() can fold 1/sqrt(dk)
WC = 257                      # conv out cols: 256 oc + 1 sum-of-oc (mean)
MAGIC = 0x5F3759DF


# ---------------------------------------------------------------------------
# device program
# ---------------------------------------------------------------------------


def build_program(b_core, ln_trivial=True):
    n_sup = b_core // SUP
    assert n_sup * SUP == b_core
    nc = bacc.Bacc("TRN2", target_bir_lowering=False, debug=False)

    x32 = nc.dram_tensor("x32", [b_core * NP, D], FP32, kind="ExternalInput")
    w_mov = nc.dram_tensor("w_mov", [3, 2, 3, 128, WC], BF16,
                           kind="ExternalInput")
    wqf_s = nc.dram_tensor("wqf_s", [2, 128, DQ], BF16, kind="ExternalInput")
    wqp_rep = nc.dram_tensor("wqp_rep", [128, H], BF16, kind="ExternalInput")
    static_f = nc.dram_tensor("static_f", [N, D], BF16, kind="ExternalInput")
    ident_sel = nc.dram_tensor("ident_sel", [N, 128], BF16,
                               kind="ExternalInput")
    id128 = nc.dram_tensor("id128", [128, 128], BF16, kind="ExternalInput")
    ones_col = nc.dram_tensor("ones_col", [1, 128], BF16,
                              kind="ExternalInput")
    bo_row = nc.dram_tensor("bo_row", [1, D], BF16, kind="ExternalInput")
    wo_m = nc.dram_tensor("wo_m", [2, 128, D], BF16, kind="ExternalInput")
    gb_rep = nc.dram_tensor("gb_rep", [6, D], BF16, kind="ExternalInput")
    y32 = nc.dram_tensor("y32", [b_core * NP, D], FP32, kind="ExternalOutput")

    with tile.TileContext(nc) as tc:
        with (
            tc.tile_pool(name="const", bufs=1) as const,
            tc.tile_pool(name="xpool", bufs=2) as xpool,
            tc.tile_pool(name="work", bufs=2) as work,
            tc.tile_pool(name="small", bufs=4) as small,
            tc.tile_pool(name="pconv", bufs=4, space="PSUM") as pconv,
            tc.tile_pool(name="pattn", bufs=4, space="PSUM") as pattn,
        ):
            C = _load_consts(nc, const, w_mov, wqf_s, wqp_rep, static_f,
                             ident_sel, id128, ones_col, bo_row, wo_m, gb_rep)
            for s in range(n_sup):
                _supertile(nc, s, C, xpool, work, small, pconv, pattn,
                           x32, y32, ln_trivial)
    nc.finalize()
    return nc


def _load_consts(nc, const, w_mov, wqf_s, wqp_rep, static_f, ident_sel,
                 id128, ones_col, bo_row, wo_m, gb_rep):
    C = {}

    def ld(name, shape, dram, in_ap=None):
        t = const.tile(shape, BF16, tag=name)
        nc.sync.dma_start(t[...], dram[...] if in_ap is None else in_ap)
        C[name] = t
        return t

    ld("w", [128, 3, 2, 3, WC], w_mov,
       w_mov[...].rearrange("p c t k o -> k p c t o"))
    ld("wqf", [128, 2, DQ], wqf_s, wqf_s[...].rearrange("c k m -> k c m"))
    ld("wqp", [128, H], wqp_rep)
    ld("stat", [N, D], static_f)
    ld("isel", [N, 128], ident_sel)
    ld("id128", [128, 128], id128)
    ld("ones", [1, 128], ones_col)
    ld("bo", [1, D], bo_row)
    ld("wo", [128, 2, D], wo_m, wo_m[...].rearrange("c k o -> k c o"))
    ld("gb", [6, D], gb_rep)
    zrow = const.tile([1, D], BF16, tag="zrow", name="zrow")
    nc.gpsimd.memset(zrow[...], 0.0)
    C["zrow"] = zrow
    magic = const.tile([128, 3], UI32, tag="magic", name="magic")
    nc.vector.memset(magic[...], MAGIC)
    C["magic"] = magic
    return C


def _supertile(nc, s, C, xpool, work, small, pconv, pattn, x32, y32,
               ln_trivial):
    # ---- phase A: load + cast x ----
    x_raw = xpool.tile([128, 4, D], FP32, tag="x_raw", name="x_raw")
    nc.sync.dma_start(
        x_raw[...],
        x32[s * SUP * NP:(s + 1) * SUP * NP, :]
        .rearrange("(g a i) d -> (a i) g d", g=4, a=4))
    x_bf = xpool.tile([128, 4, D], BF16, tag="x_bf", name="x_bf")
    nc.scalar.copy(x_bf[...], x_raw[...])

    # ---- phase B: XT (feature-major x) via PE transpose; guard columns ----
    xt = [xpool.tile([128, 514], BF16, tag=f"xt{c}", name=f"xt{c}")
          for c in range(2)]
    for c in range(2):
        nc.vector.memset(xt[c][:, 0:514:513], 0.0)
    for g in range(4):
        for c in range(2):
            ptr = pconv.tile([128, 128], BF16, tag="cbank", name="ptr")
            nc.tensor.transpose(ptr[:, 0:128],
                                x_bf[:, g:g + 1, c * 128:(c + 1) * 128],
                                C["id128"][...])
            eng = nc.vector if c == 0 else nc.scalar
            if c == 0:
                nc.vector.tensor_copy(
                    xt[c][:, 1 + g * 128:1 + (g + 1) * 128], ptr[:, 0:128])
            else:
                nc.scalar.copy(
                    xt[c][:, 1 + g * 128:1 + (g + 1) * 128], ptr[:, 0:128])

    # ---- phase C: qf for the whole supertile ----
    pqf = pconv.tile([128, 8, 32], FP32, tag="cbank", name="pqf")
    for par in range(2):
        for c in range(2):
            mv = xt[c][:, 1:513].rearrange("p (t q i) -> p t q i", q=2, i=32)
            nc.tensor.matmul(pqf[64 * par:64 * par + 64, :, :],
                             C["wqf"][:, c:c + 1, :], mv[:, :, par, :],
                             start=(c == 0), stop=(c == 1),
                             skip_group_check=True)
    pqf_sb = work.tile([128, 8, 32], BF16, tag="pqf_sb", name="pqf_sb")
    nc.vector.tensor_copy(pqf_sb[...], pqf[...])

    # ---- phase D: conv + LN (all 4 groups) ----
    qkv = [work.tile([128, 4, D], BF16, tag=f"qkv{p}", name=f"qkv{p}")
           for p in range(3)]
    c_ln = [work.tile([128, 4, D], BF16, tag=f"cln{p}", name=f"cln{p}")
            for p in range(3)]
    for g in range(4):
        _conv_ln(nc, g, C, work, small, pconv, xt, c_ln, ln_trivial)

    # ---- phase E: residual adds, batched per proj ----
    for p in range(3):
        nc.gpsimd.tensor_tensor(qkv[p][...], c_ln[p][...], x_bf[...],
                                op=ALU.add)

    # ---- phase F: per-head feature-major q/k via 32-block transposes ----
    qt_all = work.tile([128, 4, H, 32], BF16, tag="qt_all", name="qt_all")
    kt_all = work.tile([128, 4, H, 32], BF16, tag="kt_all", name="kt_all")
    nc.vector.transpose(qt_all[...],
                        qkv[0][...].rearrange("p g (h d) -> p g h d", d=32))
    nc.vector.transpose(kt_all[...],
                        qkv[1][...].rearrange("p g (h d) -> p g h d", d=32))

    # ---- phase G: scores + softmax numerator per group ----
    attn_u = work.tile([128, 4, H, 32], BF16, tag="attn_u", name="attn_u")
    den = small.tile([128, 4, H], FP32, tag="den", name="den")
    rden = small.tile([128, 4, H], FP32, tag="rden", name="rden")
    for g in range(4):
        _scores(nc, s, g, C, work, small, pconv, pattn, pqf_sb, qt_all,
                kt_all, attn_u, den, rden)

    # ---- phase H: attn transpose (whole supertile) ----
    attn_t = work.tile([128, 4, H, 32], BF16, tag="attn_t", name="attn_t")
    nc.vector.transpose(attn_t[...], attn_u[...])

    # ---- phase I: attn @ v, output projection, store ----
    y_sb = work.tile([128, 4, D], FP32, tag="y_sb", name="y_sb")
    py = None
    for g in range(4):
        py = _out_proj(nc, g, C, work, small, pconv, pattn, attn_t, rden,
                       qkv[2], py, y_sb)
    nc.sync.dma_start(
        y32[s * SUP * NP:(s + 1) * SUP * NP, :]
        .rearrange("(g a i) d -> (a i) g d", g=4, a=4),
        y_sb[...])


def _conv_ln(nc, g, C, work, small, pconv, xt, c_ln, ln_trivial):
    sS = small.tile([128, 3], FP32, tag="sS", name="sS")
    sQ = small.tile([128, 3], FP32, tag="sQ", name="sQ")
    pcs = []
    seq = [(0, 1), (0, 0), (0, 2), (1, 0), (1, 2), (1, 1)]
    for p in range(3):
        pc = pconv.tile([128, WC], FP32, tag="cbank", name=f"pc{p}")
        pcs.append(pc)
        for idx, (c, tap) in enumerate(seq):
            lhs = xt[c][:, 1 + g * 128 + (tap - 1):
                        1 + g * 128 + (tap - 1) + 128]
            nc.tensor.matmul(pc[...], lhs,
                             C["w"][:, p:p + 1, c:c + 1, tap:tap + 1, :],
                             start=(idx == 0), stop=(idx == len(seq) - 1),
                             skip_group_check=True)
        junk = small.tile([128, D], BF16, tag="junk", name="junk")
        nc.scalar.activation(junk[...], pc[:, 0:D], AF.Square,
                             accum_out=sQ[:, p:p + 1])
        nc.vector.tensor_copy(sS[:, p:p + 1], pc[:, D:D + 1])

    # stats: isig = 1/sigma, negnm = -mu/sigma  (Quake rsqrt on DVE)
    t0 = small.tile([128, 3], FP32, tag="t0", name="t0")
    nc.vector.tensor_tensor(t0[...], sS[...], sS[...], op=ALU.mult)
    vr = small.tile([128, 3], FP32, tag="vr", name="vr")
    nc.vector.tensor_scalar(vr[...], sQ[...], float(D), EPS * D * D,
                            op0=ALU.mult, op1=ALU.add)
    nc.vector.tensor_tensor(vr[...], vr[...], t0[...], op=ALU.subtract)
    sh = small.tile([128, 3], UI32, tag="sh", name="sh")
    nc.vector.tensor_scalar(sh[...], vr[...].bitcast(UI32), 1, None,
                            op0=ALU.logical_shift_right)
    y0u = small.tile([128, 3], UI32, tag="y0u", name="y0u")
    nc.vector.tensor_tensor(y0u[...], C["magic"][...], sh[...],
                            op=ALU.subtract)
    y = y0u[...].bitcast(FP32)
    for it in range(2):
        t = small.tile([128, 3], FP32, tag=f"nt{it}", name=f"nt{it}")
        nc.vector.tensor_tensor(t[...], y, y, op=ALU.mult)
        nc.vector.tensor_tensor(t[...], t[...], vr[...], op=ALU.mult)
        nc.vector.tensor_scalar(t[...], t[...], -0.5, 1.5,
                                op0=ALU.mult, op1=ALU.add)
        nc.vector.tensor_tensor(y, y, t[...], op=ALU.mult)
    isig = small.tile([128, 3], FP32, tag="isig", name="isig")
    nc.vector.tensor_scalar(isig[...], y, float(D), None, op0=ALU.mult)
    negnm = small.tile([128, 3], FP32, tag="negnm", name="negnm")
    nc.vector.tensor_tensor(negnm[...], sS[...], y, op=ALU.mult)
    nc.vector.tensor_scalar(negnm[...], negnm[...], -1.0, None, op0=ALU.mult)

    for p in range(3):
        dst = c_ln[p][:, g:g + 1, :]
        if ln_trivial:
            nc.scalar.activation(dst, pcs[p][:, 0:D].unsqueeze(1),
                                 AF.Identity, bias=negnm[:, p:p + 1],
                                 scale=isig[:, p:p + 1])
        else:
            t1 = small.tile([128, D], BF16, tag="t1", name="t1")
            nc.scalar.activation(t1[...].unsqueeze(1), pcs[p][:, 0:D].unsqueeze(1),
                                 AF.Identity, bias=negnm[:, p:p + 1],
                                 scale=isig[:, p:p + 1])
            t2 = small.tile([128, D], BF16, tag="t2", name="t2")
            nc.gpsimd.tensor_tensor(
                t2[...], t1[...],
                C["gb"][2 * p:2 * p + 1, :].broadcast_to([128, D]),
                op=ALU.mult)
            nc.gpsimd.tensor_tensor(
                dst, t2[...].unsqueeze(1),
                C["gb"][2 * p + 1:2 * p + 2, :].broadcast_to([128, D])
                .unsqueeze(1), op=ALU.add)


def _scores(nc, s, g, C, work, small, pconv, pattn, pqf_sb, qt_all, kt_all,
            attn_u, den, rden):
    # pair differences + tanh
    dd = work.tile([128, 2, N, N], BF16, tag="dd", name="dd")
    for t in range(2):
        src = pqf_sb[:, 2 * g + t:2 * g + t + 1, 0:N]
        in0 = src.unsqueeze(3).broadcast_to([128, 1, N, N])
        in1 = src.unsqueeze(2).broadcast_to([128, 1, N, N])
        eng = nc.vector if t == 0 else nc.gpsimd
        eng.tensor_tensor(dd[:, t:t + 1, :, :], in0, in1, op=ALU.subtract)
    pp = work.tile([128, 2, N, N], BF16, tag="pp", name="pp")
    nc.scalar.activation(pp[...], dd[...], AF.Tanh)

    ps = pattn.tile([128, H, 32], FP32, tag="abank", name="sbank")
    nc.tensor.matmul(ps[...], C["isel"][...], C["stat"][...],
                     start=True, stop=False, skip_group_check=True)
    for h in range(H):
        for a in range(4):
            nc.tensor.matmul(
                ps[32 * a:32 * a + N, h:h + 1, 0:N],
                qt_all[32 * a:32 * a + 32, g, h:h + 1, 0:N],
                kt_all[32 * a:32 * a + 32, g, h:h + 1, 0:N],
                start=False, stop=False, skip_group_check=True,
                tile_position=(32 * a, 32 * a))
    for j in range(N):
        for t in range(2):
            for par in range(2):
                a = 2 * t + par
                nc.tensor.matmul(
                    ps[32 * a:32 * a + N, :, j:j + 1],
                    pp[64 * par:64 * par + 64, t:t + 1, :, j:j + 1],
                    C["wqp"][64 * par:64 * par + 64, :],
                    start=False, stop=False, skip_group_check=True,
                    tile_position=(64 * par, 32 * a))
    nc.tensor.matmul(ps[...], C["ones"][...], C["zrow"][...],
                     start=False, stop=True, skip_group_check=True)

    # softmax numerator (no max subtraction) + denominator
    nc.scalar.activation(attn_u[:, g, :, :], ps[...], AF.Exp, scale=SCALE)
    nc.vector.reduce_sum(den[:, g, :], attn_u[:, g, :, 0:N],
                         axis=mybir.AxisListType.X)
    nc.vector.reciprocal(rden[:, g, :], den[:, g, :])


def _out_proj(nc, g, C, work, small, pconv, pattn, attn_t, rden, v, py,
              y_sb):
    po = pattn.tile([128, H, 32], FP32, tag="abank", name="obank")
    for h in range(H):
        for a in range(4):
            nc.tensor.matmul(
                po[32 * a:32 * a + 32, h:h + 1, :],
                attn_t[32 * a:32 * a + 32, g, h:h + 1, :],
                v[32 * a:32 * a + 32, g:g + 1, 32 * h:32 * h + 32],
                skip_group_check=True,
                tile_position=(32 * a, 32 * a))
    o_bf = work.tile([128, H, 32], BF16, tag="o_bf", name="o_bf")
    nc.vector.tensor_tensor(
        o_bf[...], po[...],
        rden[:, g, :].unsqueeze(2).broadcast_to([128, H, 32]), op=ALU.mult)
    # O row-major [(a,i), (h,d)] -> feature-major via PE transpose
    ot_bf = [work.tile([128, 128], BF16, tag=f"ot{c}", name=f"ot{c}")
             for c in range(2)]
    for c in range(2):
        ptc = pconv.tile([128, 128], BF16, tag="cbank", name="ptc")
        nc.tensor.transpose(
            ptc[...],
            o_bf[:, 4 * c:4 * (c + 1), :].rearrange("p h d -> p (h d)"),
            C["id128"][...])
        if c == 0:
            nc.vector.tensor_copy(ot_bf[c][...], ptc[...])
        else:
            nc.scalar.copy(ot_bf[c][...], ptc[...])

    slot = g % 2
    if slot == 0:
        py = pattn.tile([128, 2, D], FP32, tag="abank", name="ybank")
    nc.tensor.matmul(py[:, slot:slot + 1, :], C["ones"][...],
                     C["bo"][...], start=True, stop=False,
                     skip_group_check=True)
    for c in range(2):
        nc.tensor.matmul(py[:, slot:slot + 1, :], ot_bf[c][...],
                         C["wo"][:, c:c + 1, :],
                         start=False, stop=(c == 1),
                         skip_group_check=True)
    if slot == 1:
        dst = y_sb[:, g - 1:g + 1, :]
        if g == 1:
            nc.vector.tensor_copy(dst, py[...])
        else:
            nc.scalar.copy(dst, py[...])
    return py


# ---------------------------------------------------------------------------
# host wrapper
# ---------------------------------------------------------------------------

_CACHE = {}


def _prep_consts(wq, wk, wv, rel_table, global_bias, alpha, wqf, wqp, bqp,
                 wo, bo, ln_g, ln_b):
    bf = np.float32  # cast to bf16 via ml_dtypes at the end
    import ml_dtypes
    b16 = ml_dtypes.bfloat16

    w_mov = np.zeros([3, 2, 3, 128, WC], bf)
    for p, w in enumerate((wq, wk, wv)):
        for c in range(2):
            for t in range(3):
                wt = w[:, 128 * c:128 * (c + 1), t].T  # [128 in, 256 oc]
                w_mov[p, c, t, :, 0:D] = wt
                w_mov[p, c, t, :, D] = wt.sum(axis=1)
    wqf_s = np.stack([wqf[0:128], wqf[128:256]])
    wqp_rep = np.tile(wqp * RS, (2, 1))

    ids = np.arange(N)
    rel_idx = ids[:, None] - ids[None, :] + N - 1
    rel_bias = rel_table[rel_idx].transpose(2, 0, 1)          # [H, N, N]
    static = (rel_bias + global_bias * np.float32(alpha)) * RS
    static_f = np.full([N, D], -3000.0, bf)
    for h in range(H):
        static_f[:, 32 * h:32 * h + N] = static[h]            # [i, (h, j)]
    if np.any(bqp != 0):
        for h in range(H):
            static_f[:, 32 * h:32 * h + N] += bqp[h] * RS

    ident_sel = np.zeros([N, 128], bf)
    for a in range(4):
        ident_sel[np.arange(N), 32 * a + np.arange(N)] = 1.0
    id128 = np.eye(128, dtype=bf)
    ones_col = np.ones([1, 128], bf)
    bo_row = bo.reshape(1, D)
    wo_m = np.stack([wo[0:128], wo[128:256]])
    gb_rep = np.stack([ln_g[0], ln_b[0], ln_g[1], ln_b[1], ln_g[2], ln_b[2]])

    consts = dict(w_mov=w_mov, wqf_s=wqf_s, wqp_rep=wqp_rep,
                  static_f=static_f, ident_sel=ident_sel, id128=id128,
                  ones_col=ones_col, bo_row=bo_row, wo_m=wo_m, gb_rep=gb_rep)
    return {k: np.asarray(v, b16) for k, v in consts.items()}


def kernel(x, wq, wk, wv, ln_q_g, ln_q_b, ln_k_g, ln_k_b, ln_v_g, ln_v_b,
           rel_table, global_bias, alpha, wqf, bqf, wqp, bqp, wo, bo):
    x = np.asarray(x, np.float32)
    B = x.shape[0]
    b_core = B // N_CORES
    assert np.allclose(np.asarray(bqf), 0.0), "bqf folds out of pair diffs"

    ln_g = [np.asarray(v, np.float32) for v in (ln_q_g, ln_k_g, ln_v_g)]
    ln_b = [np.asarray(v, np.float32) for v in (ln_q_b, ln_k_b, ln_v_b)]
    ln_trivial = all(np.allclose(g, 1.0) for g in ln_g) and \
        all(np.allclose(b, 0.0) for b in ln_b)

    key = (b_core, ln_trivial)
    if key not in _CACHE:
        _CACHE[key] = build_program(b_core, ln_trivial)
    nc = _CACHE[key]

    consts = _prep_consts(
        np.asarray(wq, np.float32), np.asarray(wk, np.float32),
        np.asarray(wv, np.float32), np.asarray(rel_table, np.float32),
        np.asarray(global_bias, np.float32), np.float32(alpha),
        np.asarray(wqf, np.float32), np.asarray(wqp, np.float32),
        np.asarray(bqp, np.float32), np.asarray(wo, np.float32),
        np.asarray(bo, np.float32), ln_g, ln_b)

    # 32-pad positions, shard across cores
    x32 = np.zeros([B, NP, D], np.float32)
    x32[:, :N] = x
    x32 = x32.reshape(N_CORES, b_core * NP, D)

    in_maps = []
    for cidx in range(N_CORES):
        m = {"x32": x32[cidx]}
        m.update(consts)
        in_maps.append(m)

    run = _get_runner(key, nc)
    outs = run(in_maps)
    outs = [o.reshape(b_core, NP, D)[:, :N] for o in outs]
    return np.concatenate(outs, axis=0).astype(np.float32)


_RUNNERS = {}


def _get_runner(key, nc):
    """Build (once) a jitted 8-core SPMD executor for `nc`."""
    if key in _RUNNERS:
        return _RUNNERS[key]
    import jax
    from jax.sharding import Mesh, PartitionSpec
    from jax.experimental.shard_map import shard_map
    from concourse import bass2jax, mybir as mb

    bass2jax.install_neuronx_cc_hook()
    partition_name = (nc.partition_id_tensor.name
                      if nc.partition_id_tensor else None)
    in_names, out_names, out_avals, zero_outs = [], [], [], []
    for alloc in nc.m.functions[0].allocations:
        if not isinstance(alloc, mb.MemoryLocationSet):
            continue
        name = alloc.memorylocations[0].name
        if alloc.kind == "ExternalInput":
            if name != partition_name:
                in_names.append(name)
        elif alloc.kind == "ExternalOutput":
            shape = tuple(alloc.tensor_shape)
            dtype = mb.dt.np(alloc.dtype)
            out_names.append(name)
            out_avals.append(jax.core.ShapedArray(shape, dtype))
            zero_outs.append(np.zeros(shape, dtype))
    n_params = len(in_names)
    all_names = list(in_names) + list(out_names)
    if partition_name is not None:
        all_names.append(partition_name)

    def _body(*args):
        operands = list(args)
        if partition_name is not None:
            operands.append(bass2jax.partition_id_tensor())
        return tuple(bass2jax._bass_exec_p.bind(
            *operands, out_avals=tuple(out_avals),
            in_names=tuple(all_names), out_names=tuple(out_names),
            lowering_input_output_aliases=(), sim_require_finite=True,
            sim_require_nnan=True, nc=nc))

    devices = jax.devices()[:N_CORES]
    mesh = Mesh(np.asarray(devices), ("core",))
    n_ops = n_params + len(out_names)
    fn = jax.jit(shard_map(
        _body, mesh=mesh, in_specs=(PartitionSpec("core"),) * n_ops,
        out_specs=(PartitionSpec("core"),) * len(out_names),
        check_rep=False), keep_unused=True)

    def run(in_maps, timing_reps=0):
        import jax as _jax
        concat = []
        for i, name in enumerate(in_names):
            concat.append(np.concatenate(
                [np.asarray(in_maps[c][name]) for c in range(N_CORES)],
                axis=0))
        concat += [np.zeros((N_CORES * z.shape[0], *z.shape[1:]), z.dtype)
                   for z in zero_outs]
        out = fn(*concat)
        _jax.block_until_ready(out)
        if timing_reps:
            dev = [_jax.device_put(a) for a in concat]
            _jax.block_until_ready(dev)
            best = float("inf")
            import time as _t
            for _ in range(timing_reps):
                t0 = _t.perf_counter()
                o = fn(*dev)
                _jax.block_until_ready(o)
                best = min(best, _t.perf_counter() - t0)
            run.last_exec_s = best
        y = np.asarray(out[0])
        per = y.shape[0] // N_CORES
        return [y[c * per:(c + 1) * per] for c in range(N_CORES)]

    _RUNNERS[key] = run
    return run


# ---------------------------------------------------------------------------
# dev: simulator check on a small shard
# ---------------------------------------------------------------------------

def _patch_sim_strided_matmul():
    """CoreSim's PSUM zero-region model only handles contiguous matmul
    out free-APs.  For strided outs (our dyn-bias accumulation) the HW
    per-element has_written semantics reduce to pure accumulation, since
    the bytes were started+written by the preceding full-tile matmul."""
    import concourse.bass_interp as bi
    import numpy as _np
    cls = bi.InstructionExecutor
    if getattr(cls, "_strided_mm_patched", False):
        return
    orig = cls.visit_InstMatmult

    def visit(self, instruction, *, reg_snapshot=None):
        out = instruction.outs[0]
        ap = [d for d in out.ap[1:] if d[1] > 1]
        contig = (not ap) or ap[-1][0] == 1
        if contig:
            return orig(self, instruction, reg_snapshot=reg_snapshot)
        assert not instruction.start_tensor_calc
        assert not instruction.stop_tensor_calc
        from concourse.bass_interp import Direction
        ifmap = self.view_ap(instruction.ins[0], Direction.READ, instruction,
                             reg_snapshot=reg_snapshot)
        weights = self.view_ap(instruction.ins[1], Direction.READ,
                               instruction, reg_snapshot=reg_snapshot)
        out_view = self.view_ap(out, Direction.WRITE, instruction,
                                required_byte_align=4,
                                reg_snapshot=reg_snapshot)
        i = ifmap.astype(_np.float32).reshape(ifmap.shape[0], -1)
        w = weights.astype(_np.float32).reshape(weights.shape[0], -1)
        out_view[:] += (w.T @ i).reshape(out_view.shape)

    cls.visit_InstMatmult = visit
    cls._strided_mm_patched = True


def _sim_check(b_core=16):
    _patch_sim_strided_matmul()
    from concourse.bass_interp import CoreSim
    sys.path.insert(0, "/root/problem")
    import reference

    inp = {k: np.asarray(v) for k, v in reference.setup_inputs().items()}
    xs = inp["x"][:b_core]
    ln_g = [inp[f"ln_{p}_g"] for p in "qkv"]
    ln_b = [inp[f"ln_{p}_b"] for p in "qkv"]
    nc = build_program(b_core, True)
    consts = _prep_consts(inp["wq"], inp["wk"], inp["wv"], inp["rel_table"],
                          inp["global_bias"], np.float32(inp["alpha"]),
                          inp["wqf"], inp["wqp"], inp["bqp"], inp["wo"],
                          inp["bo"], ln_g, ln_b)
    x32 = np.zeros([b_core, NP, D], np.float32)
    x32[:, :N] = xs
    sim = CoreSim(nc)
    sim.tensor("x32")[:] = x32.reshape(b_core * NP, D)
    for k, v in consts.items():
        sim.tensor(k)[:] = v
    sim.simulate()
    y = np.asarray(sim.tensor("y32")).reshape(b_core, NP, D)[:, :N]

    ref_in = dict(inp)
    ref_in["x"] = xs
    exp = np.asarray(reference.reference(**ref_in))
    rel = np.linalg.norm(y - exp) / np.linalg.norm(exp)
    print("sim rel err:", rel)
    return rel


if __name__ == "__main__":
    _sim_check(int(os.environ.get("SIM_B", "16")))
